# revision 1
# baseline (speedup 1.0000x reference)
"""DeepConvNet Trainium2 kernel.

3x [Conv3x3(pad=1) -> ReLU -> MaxPool2x2] -> Linear, N=64, input 3x128x128.

Sharding: pure data parallel, 8 images per NeuronCore across 8 cores.

Per-core dataflow (activations bf16 in SBUF, fp32 PSUM accumulation):
  conv1: im2col in partitions. 4-image groups, block-diagonal weights:
         K = 9 taps x 3 ch x 4 imgs (+1 bias row) = 109 partitions,
         M = 4 imgs x 32 ch. rhs built by 27 strided DMAs from
         host-padded x in HBM (9 taps x 3 column chunks, each covering
         both groups); chunks gate conv1 k-blocks so matmuls start as
         soon as the first 1.8MB lands.  Warmup matmuls run on a memset
         tile (no DMA dependency) so the PE is busy and the HAM clock
         gate warms right after the NEFF preamble.
  conv2: DIRECT from pp1 -- no kx-replicated rhs DMA at all (the 6.7MB
         SBUF->SBUF replication was nearly half of all DMA traffic and
         serialized pool1 -> DMA -> conv2).  9 accumulated matmuls
         (K=64 = 2 imgs x 32 ch block-diagonal, M=128 = 2 imgs x 64 F)
         read tap-shifted windows of pp1 in place; the two image pairs
         of a group run CONCURRENTLY via PE row tiling (pair A rows
         0-63, pair B rows 64-127), like conv3.  Bias+ReLU fold into
         the pool evacuation (two scalar-engine activations with
         per-partition bias, then DVE maxes).
  conv3: 9 accumulated matmuls (K=64) per image; two images run
         concurrently via row tiling (A rows 0-63, B 64-127). Bias+ReLU
         fold into the pool evacuation like conv2, so no serial
         activation pass sits between conv3 and the fc.
  pool:  PSUM can only feed one operand of a DVE op, so ScalarE copies
         even columns PSUM->SBUF (applying bias+ReLU where folded), DVE
         maxes odd PSUM columns against the copy, then maxes row pairs
         into a zero-bordered padded tile.
  fc:    256 accumulated matmuls (K=128 channels, one per spatial p),
         N = 8 images, M = 10 classes, 4-way column tiling.
"""

import os
import sys

import numpy as np

for _p in ("/opt/trn_rl_repo", "/root/.axon_site/_ro/trn_rl_repo"):
    if os.path.isdir(_p) and _p not in sys.path:
        sys.path.insert(0, _p)

import ml_dtypes

import concourse.bass as bass
import concourse.mybir as mybir
import concourse.tile as tile
from concourse import bacc
from concourse.bass_utils import run_bass_kernel_spmd

BF16 = mybir.dt.bfloat16
F32 = mybir.dt.float32
NPBF16 = ml_dtypes.bfloat16

N_CORES = 8
IMGS = 8          # images per core
GROUPS = 2        # conv1 image groups per core (4 imgs each)
G1 = 130          # conv1 padded width/height
W1WIN = 127 * G1 + 128  # flat window length per conv1 im2col row
W1ALLOC = 128 * G1
P1 = 66           # conv1 pooled padded grid (64 + 2)
P1F = 67 * 66     # pp1 alloc free size (one guard row)
P2 = 34           # conv2 pooled padded grid (32 + 2)
P2F = 34 * 34
WARMUP_MMS = 90   # keep PE busy (and HAM warm) until im2col chunk 0 lands

# conv1 im2col column chunks: chunk c gates conv1 k-iterations
# [4c, 4c+4) (chunk boundary 32*c rows of the 130-pitch window).
CH = [0, 4160, 8320, W1WIN]


def _build_nc(dbg=False):
    nc = bacc.Bacc("TRN2", target_bir_lowering=False, debug=False)

    xp = nc.dram_tensor("xp", [IMGS * 3 * G1 * G1], BF16, kind="ExternalInput")
    lhsT1 = nc.dram_tensor("lhsT1", [109, 128], BF16, kind="ExternalInput")
    wl2d = nc.dram_tensor("wl2d", [128, 1152], BF16, kind="ExternalInput")
    wrest = nc.dram_tensor("wrest", [128, 3712], BF16, kind="ExternalInput")
    wf32 = nc.dram_tensor("wf32", [128, 3], F32, kind="ExternalInput")
    ones_d = nc.dram_tensor("ones_d", [2 * W1ALLOC], BF16, kind="ExternalInput")
    scores = nc.dram_tensor("scores", [10, 8], F32, kind="ExternalOutput")

    Relu = mybir.ActivationFunctionType.Relu
    Copy = mybir.ActivationFunctionType.Copy
    MAX = mybir.AluOpType.max

    with tile.TileContext(nc) as tc:
        with (
            tc.tile_pool(name="wts", bufs=1) as wp,
            tc.tile_pool(name="rhs1", bufs=1) as rhs1p,
            tc.tile_pool(name="pp1", bufs=2) as pp1p,
            tc.tile_pool(name="pp2", bufs=4) as pp2p,
            tc.tile_pool(name="xall", bufs=1) as xallp,
            tc.tile_pool(name="tmp", bufs=6) as tmpp,
            tc.tile_pool(name="ps", bufs=4, space="PSUM") as psp,
        ):
            # ---- warmup: junk matmuls with no DMA dependency
            t_warm = wp.tile([128, 128], BF16)
            nc.gpsimd.memset(t_warm[:], 0)
            ps_warm = psp.tile([128, 128], F32, tag="ps", name="ps_warm")
            for _ in range(WARMUP_MMS):
                nc.tensor.matmul(
                    ps_warm[:], t_warm[:], t_warm[:], start=True, stop=True
                )

            # ---- padded pool-output tiles: border memsets run early so
            # they never sit behind DMA waits in the gpsimd queue.
            pp1_tiles = []
            for g in range(GROUPS):
                pp1 = pp1p.tile([128, P1F], BF16, tag="pp1", name=f"pp1_{g}")
                pv = pp1.rearrange("p (r q) -> p r q", q=P1)
                nc.gpsimd.memset(pp1[:, 0:P1], 0)
                nc.gpsimd.memset(pp1[:, 65 * P1 : P1F], 0)  # bottom + guard
                nc.gpsimd.memset(pv[:, 1:65, 0:1], 0)
                nc.gpsimd.memset(pv[:, 1:65, 65:66], 0)
                pp1_tiles.append(pp1)
            pp2_tiles = []
            for q in range(4):
                pp2 = pp2p.tile([128, P2F], BF16, tag="pp2", name=f"pp2_{q}")
                pv2 = pp2.rearrange("p (r q) -> p r q", q=P2)
                nc.gpsimd.memset(pp2[:, 0:P2], 0)
                nc.gpsimd.memset(pp2[:, 33 * P2 : P2F], 0)
                nc.gpsimd.memset(pv2[:, 1:33, 0:1], 0)
                nc.gpsimd.memset(pv2[:, 1:33, 33:34], 0)
                pp2_tiles.append(pp2)

            # ---- weight / constant loads.
            # sync: lhsT1 then its im2col taps, then the late weights.
            # scalar: wf32, ones row, l2d (needed by ~conv2 start), taps.
            # gpsimd: taps only (its ring also carries nothing else big).
            t_l1 = wp.tile([109, 128], BF16)
            nc.sync.dma_start(out=t_l1[:], in_=lhsT1.ap())
            t_wf32 = wp.tile([128, 3], F32)
            nc.scalar.dma_start(out=t_wf32[:], in_=wf32.ap())
            t_b3 = t_wf32[:, 0:1]
            t_bfc = t_wf32[0:10, 1:2]
            t_b2 = t_wf32[:, 2:3]

            rhs1 = rhs1p.tile([109, 2 * W1ALLOC], BF16, name="rhs1")
            r1pitch = rhs1.ap[0][0]
            nc.scalar.dma_start(out=rhs1[0:1, :], in_=ones_d.ap())  # bias ones-row

            t_l2d = wp.tile([128, 1152], BF16)
            nc.scalar.dma_start(out=t_l2d[:], in_=wl2d.ap())

            # ---- im2col DMAs: per (chunk, tap) one DMA covering both
            # groups (24 stride-9 partition lines); round-robin queues.
            dmas = [nc.sync, nc.scalar, nc.gpsimd]
            for ci in range(len(CH) - 1):
                c0 = CH[ci]
                wlen = CH[ci + 1] - c0
                for t in range(9):
                    a, b = divmod(t, 3)
                    src = bass.AP(
                        xp,
                        a * G1 + b + c0,
                        [[G1 * G1, 12], [4 * 3 * G1 * G1, 2], [1, wlen]],
                    )
                    dst = bass.AP(
                        rhs1.tensor,
                        rhs1.offset + (1 + t) * r1pitch + c0,
                        [[9 * r1pitch, 12], [W1ALLOC, 2], [1, wlen]],
                    )
                    dmas[t % 3].dma_start(out=dst, in_=src)

            # late weights (conv3 + fc), on sync after its taps
            t_wrest = wp.tile([128, 3712], BF16)
            nc.sync.dma_start(out=t_wrest[:], in_=wrest.ap())
            t_l3 = t_wrest[:, 0:1152]
            t_wfc = t_wrest[:, 1152:3712]

            if dbg:
                d_rhs1 = nc.dram_tensor(
                    "d_rhs1", [109, 2 * W1ALLOC], BF16, kind="ExternalOutput"
                )
                nc.sync.dma_start(out=d_rhs1.ap(), in_=rhs1[:])

            x_all = xallp.tile([128, 2048], BF16)

            def pool_psum(ps, out_ap, w, name, relu):
                """2x2 maxpool of a [128, 1024] psum tile (rows of width w)
                into out_ap; relu=True also clamps at 0 (bias already in
                psum via the matmul ones-row)."""
                psv = ps.rearrange("p (a two) -> p a two", two=2)
                cp = tmpp.tile([128, 512], F32, tag="tmpc", name=f"cp_{name}")
                nc.scalar.activation(cp[:], psv[:, :, 0], Copy)
                m1 = tmpp.tile([128, 512], BF16, tag="tmpm", name=f"m1_{name}")
                nc.vector.tensor_max(m1[:], psv[:, :, 1], cp[:])
                tv = m1.rearrange("p (y two x) -> p y two x", two=2, x=w // 2)
                if relu:
                    nc.vector.scalar_tensor_tensor(
                        out_ap, tv[:, :, 0, :], 0.0, tv[:, :, 1, :], MAX, MAX
                    )
                else:
                    nc.vector.tensor_max(out_ap, tv[:, :, 0, :], tv[:, :, 1, :])

            def pool_psum_bias_relu(ps, out_ap, w, name, bias):
                """Pool with per-partition bias + ReLU folded into the two
                scalar-engine PSUM evacuations (bias/relu commute with max)."""
                psv = ps.rearrange("p (a two) -> p a two", two=2)
                cp = tmpp.tile([128, 512], F32, tag="tmpc", name=f"cpe_{name}")
                nc.scalar.activation(cp[:], psv[:, :, 0], Relu, bias=bias)
                cp2 = tmpp.tile([128, 512], F32, tag="tmpd", name=f"cpo_{name}")
                nc.scalar.activation(cp2[:], psv[:, :, 1], Relu, bias=bias)
                m1 = tmpp.tile([128, 512], BF16, tag="tmpm", name=f"m1_{name}")
                nc.vector.tensor_max(m1[:], cp2[:], cp[:])
                tv = m1.rearrange("p (y two x) -> p y two x", two=2, x=w // 2)
                nc.vector.tensor_max(out_ap, tv[:, :, 0, :], tv[:, :, 1, :])

            # =======================  conv1  =======================
            rhs1g = rhs1.rearrange("p (g y x) -> p g y x", g=2, x=G1)
            for g in range(GROUPS):
                rhs1v = rhs1g[:, g]
                pv = pp1_tiles[g].rearrange("p (r q) -> p r q", q=P1)
                for k in range(16):
                    ps = psp.tile([128, 1024], F32, tag="ps", name=f"ps1_{g}_{k}")
                    for h in range(2):
                        y0 = k * 8 + h * 4
                        nc.tensor.matmul(
                            ps[:, h * 512 : (h + 1) * 512],
                            t_l1[:],
                            rhs1v[:, y0 : y0 + 4, 0:128],
                            start=True,
                            stop=True,
                        )
                    Y0 = k * 4
                    pool_psum(
                        ps, pv[:, Y0 + 1 : Y0 + 5, 1:65], 128, f"c1_{g}_{k}", True
                    )

            # =======================  conv2 (direct from pp1)  =======================
            def conv2_group(g):
                """Both pairs of group g run concurrently: pair A = imgs
                4g+0,4g+1 (pp1 rows 0-63, PE rows 0-63), pair B = imgs
                4g+2,4g+3 (rows 64-127).  9 accumulated taps read
                tap-shifted pp1 windows in place."""
                pv = pp1_tiles[g].rearrange("p (r q) -> p r q", q=P1)
                for k in range(4):
                    ps_ab = [
                        psp.tile([128, 1024], F32, tag="ps", name=f"ps2_{g}_{k}_{jj}")
                        for jj in range(2)
                    ]
                    for h in range(2):
                        Y0 = k * 16 + h * 8
                        for t in range(9):
                            a, b = divmod(t, 3)
                            for j in range(2):  # pair A rows 0-63, pair B 64-127
                                nc.tensor.matmul(
                                    ps_ab[j][:, h * 512 : (h + 1) * 512],
                                    t_l2d[64 * j : 64 * j + 64, t * 128 : (t + 1) * 128],
                                    pv[64 * j : 64 * j + 64, Y0 + a : Y0 + a + 8, b : b + 64],
                                    start=(t == 0),
                                    stop=(t == 8),
                                )
                    for j in range(2):
                        q = 2 * g + j
                        pv2 = pp2_tiles[q].rearrange("p (r q) -> p r q", q=P2)
                        Y0 = k * 8
                        pool_psum_bias_relu(
                            ps_ab[j], pv2[:, Y0 + 1 : Y0 + 9, 1:33], 64,
                            f"c2_{q}_{k}", t_b2,
                        )

            def conv3_pair(q):
                pv2 = pp2_tiles[q].rearrange("p (r q) -> p r q", q=P2)
                ps_ab = [
                    psp.tile([128, 1024], F32, tag="ps", name=f"ps3_{q}_{jj}")
                    for jj in range(2)
                ]
                for h in range(2):
                    Y0 = h * 16
                    for t in range(9):
                        a, b = divmod(t, 3)
                        for j in range(2):  # img A (rows 0-63), img B (rows 64-127)
                            nc.tensor.matmul(
                                ps_ab[j][:, h * 512 : (h + 1) * 512],
                                t_l3[64 * j : 64 * j + 64, t * 128 : (t + 1) * 128],
                                pv2[64 * j : 64 * j + 64, Y0 + a : Y0 + a + 16, b : b + 32],
                                start=(t == 0),
                                stop=(t == 8),
                            )
                for j in range(2):
                    img = 2 * q + j
                    xv = x_all.rearrange("p (i q) -> p i q", q=256)
                    ov = xv[:, img, :].rearrange("p (y x) -> p y x", x=16)
                    pool_psum_bias_relu(ps_ab[j], ov, 32, f"c3_{q}_{j}", t_b3)

            conv2_group(0)
            conv3_pair(0)
            conv3_pair(1)
            conv2_group(1)
            conv3_pair(2)
            conv3_pair(3)

            if dbg:
                d_pp1 = nc.dram_tensor("d_pp1", [128, P1F], BF16, kind="ExternalOutput")
                nc.sync.dma_start(out=d_pp1.ap(), in_=pp1_tiles[0][:])
                d_pp2 = nc.dram_tensor("d_pp2", [128, P2F], BF16, kind="ExternalOutput")
                nc.sync.dma_start(out=d_pp2.ap(), in_=pp2_tiles[0][:])
                d_xall = nc.dram_tensor("d_xall", [128, 2048], BF16, kind="ExternalOutput")
                nc.sync.dma_start(out=d_xall.ap(), in_=x_all[:])

            # =======================  fc  =======================
            ps_fc = psp.tile([128, 8], F32, tag="ps", name="ps_fc")
            xv = x_all.rearrange("p (i q) -> p i q", q=256)
            for p in range(256):
                cg = p % 4
                nc.tensor.matmul(
                    ps_fc[32 * cg : 32 * cg + 10, :],
                    t_wfc[:, 10 * p : 10 * p + 10],
                    xv[:, :, p],
                    start=(p < 4),
                    stop=(p >= 252),
                    tile_position=(0, 32 * cg),
                )
            sc0 = wp.tile([10, 8], F32)
            nc.scalar.activation(sc0[:], ps_fc[0:10, :], Copy)
            sc1 = wp.tile([10, 8], F32)
            nc.vector.tensor_add(sc1[:], ps_fc[32:42, :], sc0[:])
            sc2 = wp.tile([10, 8], F32)
            nc.vector.tensor_add(sc2[:], ps_fc[64:74, :], sc1[:])
            sc3 = wp.tile([10, 8], F32)
            nc.vector.tensor_add(sc3[:], ps_fc[96:106, :], sc2[:])
            sc = wp.tile([10, 8], F32)
            nc.scalar.activation(
                sc[:], sc3[:], mybir.ActivationFunctionType.Identity, bias=t_bfc
            )
            nc.sync.dma_start(out=scores.ap(), in_=sc[:])

    nc.compile()
    return nc


def _prep_weights(w1, b1, w2, b2, w3, b3, w_fc, b_fc):
    """Host-side weight rearrangement (shared across cores)."""
    # conv1 block-diagonal lhsT: row 1 + (img*3+c)*9 + t, col m = img*32
    # + f; row 0 carries the bias (rhs ones-row).
    l1 = np.zeros((109, 128), np.float32)
    for t in range(9):
        a, b = divmod(t, 3)
        for img in range(4):
            for c in range(3):
                l1[1 + (img * 3 + c) * 9 + t, img * 32 : img * 32 + 32] = w1[:, c, a, b]
    l1[0, :] = np.tile(np.asarray(b1, np.float32), 4)
    # conv2 direct: per tap t a [128, 128] block: rows 0-31 (img-even ch)
    # -> cols 0-63 (img-even F), rows 32-63 (img-odd ch) -> cols 64-127;
    # rows 64-127 duplicate rows 0-63 (pair B at PE rows 64-127).
    l2d = np.zeros((128, 9 * 128), np.float32)
    for t in range(9):
        a, b = divmod(t, 3)
        blk = w2[:, :, a, b].T  # [c=32, f=64]
        l2d[0:32, t * 128 : t * 128 + 64] = blk
        l2d[32:64, t * 128 + 64 : t * 128 + 128] = blk
    l2d[64:128, :] = l2d[0:64, :]
    # conv3: rows c (dup at 64+c), col block t
    l3 = np.zeros((128, 9 * 128), np.float32)
    for t in range(9):
        a, b = divmod(t, 3)
        blk = w3[:, :, a, b].T  # [c=64, f=128]
        l3[0:64, t * 128 : (t + 1) * 128] = blk
        l3[64:128, t * 128 : (t + 1) * 128] = blk
    # fc: w_fc[c*256 + p, cls] -> wfc[c, p*10 + cls]
    wf = np.ascontiguousarray(w_fc.reshape(128, 256, 10).reshape(128, 2560))
    wrest = np.concatenate([l3, wf], axis=1)
    wf32 = np.zeros((128, 3), np.float32)
    wf32[:, 0] = np.asarray(b3, np.float32)
    wf32[0:10, 1] = np.asarray(b_fc, np.float32)
    wf32[:, 2] = np.tile(np.asarray(b2, np.float32), 2)
    return {
        "lhsT1": l1.astype(NPBF16),
        "wl2d": l2d.astype(NPBF16),
        "wrest": wrest.astype(NPBF16),
        "wf32": wf32,
        "ones_d": np.ones(2 * W1ALLOC, NPBF16),
    }


_NC_CACHE = {}


def get_nc():
    if "nc" not in _NC_CACHE:
        _NC_CACHE["nc"] = _build_nc()
    return _NC_CACHE["nc"]


def kernel(x, w1, b1, w2, b2, w3, b3, w_fc, b_fc, **run_kwargs):
    x = np.asarray(x, np.float32)
    wts = _prep_weights(
        np.asarray(w1, np.float32), np.asarray(b1, np.float32),
        np.asarray(w2, np.float32), np.asarray(b2, np.float32),
        np.asarray(w3, np.float32), np.asarray(b3, np.float32),
        np.asarray(w_fc, np.float32), np.asarray(b_fc, np.float32),
    )
    xpad = np.pad(x, ((0, 0), (0, 0), (1, 1), (1, 1))).astype(NPBF16)
    in_maps = []
    for core in range(N_CORES):
        m = dict(wts)
        m["xp"] = np.ascontiguousarray(xpad[core * IMGS : (core + 1) * IMGS]).reshape(-1)
        in_maps.append(m)

    nc = get_nc()
    res = run_bass_kernel_spmd(nc, in_maps, core_ids=list(range(N_CORES)), **run_kwargs)
    out = np.concatenate([r["scores"].T for r in res.results], axis=0)
    kernel.last_results = res
    return out.astype(np.float32)

